# revision 16
# baseline (speedup 1.0000x reference)
"""CharLevelEncoder Trainium2 kernel (8-core SPMD).

Math: out = relu(concat(word_emb[word_ids], h(char_ids)) @ W_lin.T + b_lin)
with h a single LSTM cell step from zero state on E[char_ids].

Key algebraic restructuring:
  - h depends only on char_id (40 values) -> HB table [40, WD]:
        HB[c] = h_c @ W_lin[:, WD:].T + b_lin
  - word_emb[word_ids] @ W_lin[:, :WD].T == (word_emb @ W_lin[:, :WD].T)[word_ids]
    so precompute P = word_emb @ A.T once (68.7 GFLOP total, sharded by vocab)
  - out[t] = relu(P[word_ids[t]] + HB[char_ids[t]])

Device program per core (identical across cores, data differs):
  - words are host-packed into bins of <=128 words and <=512 chars; each bin is
    one "chunk": P_chunk [128, WD] = weT_chunk.T @ A.T via 16 matmuls
  - each chunk's chars form up to T_c tiles of 128 chars; per tile, a one-hot
    select matrix gathers P rows via PE matmul, a one-hot char-id matrix adds
    HB rows into the same PSUM, ACT applies relu, DMA stores the rows.
  - host routes chars to the core owning their word's bin and inverse-permutes
    the output.
"""

import os

import ml_dtypes
import numpy as np

import concourse.bass as bass
import concourse.tile as tile
from concourse import bacc, mybir
from concourse.bass_utils import run_bass_kernel_spmd

NCORES = 8
WD = 1024
NE = 40
HID = 512
BIN_CHAR_CAP = 512
BIN_WORD_CAP = 128

# compute dtype for matmul operands: bf16 runs 1 cyc/row on PE (f32r measured
# 2 cyc/row); simulated end-to-end rel err 2.6e-3 (bf16) vs 1.6e-4 (f32r).
USE_F32R = os.environ.get("KERNEL_F32R", "0") == "1"
MM_DT = mybir.dt.float32r if USE_F32R else mybir.dt.bfloat16
NP_DT = np.float32 if USE_F32R else ml_dtypes.bfloat16


def _sigmoid(x):
    return 1.0 / (1.0 + np.exp(-x))


def _hb_table(E, W_ih, b_ih, b_hh, W_lin, b_lin):
    G = E.astype(np.float32) @ W_ih.T + b_ih + b_hh  # [NE, 4H]
    i, f, g, o = np.split(G, 4, axis=1)
    c = _sigmoid(i) * np.tanh(g)
    h = _sigmoid(o) * np.tanh(c)  # [NE, H]
    return (h @ W_lin[:, WD:].T + b_lin).astype(np.float32)  # [NE, WD]


def _pack_bins(word_ids, n_words):
    """Pack words into bins of <=BIN_WORD_CAP words and <=BIN_CHAR_CAP chars.
    Snake-deal of words sorted by char count keeps bin char totals equal."""
    wc = np.bincount(word_ids, minlength=n_words)
    used = np.nonzero(wc)[0]
    counts = wc[used]
    assert counts.max() <= BIN_CHAR_CAP, "single word exceeds bin capacity"
    order = np.argsort(-counts, kind="stable")
    wsorted = used[order]
    csorted = counts[order]
    nused = len(used)
    total = int(counts.sum())

    # First-fit-decreasing with exact-fill preference: fill bins to exactly
    # BIN_CHAR_CAP chars so tiles (128 chars each) waste minimal padding.
    # Words are sorted by count desc; maintain open bins keyed by remaining
    # char capacity; close bins when word slots run out.
    bin_of = np.empty(nused, np.int32)
    rem_chars = []  # per-bin remaining char capacity
    rem_words = []  # per-bin remaining word slots
    open_bins = []  # indices of bins with capacity
    for j in range(nused):
        c = int(csorted[j])
        placed = -1
        # first open bin that fits (open_bins kept in creation order; counts
        # are descending so early bins fill tightest)
        for b in open_bins:
            if rem_chars[b] >= c and rem_words[b] > 0:
                placed = b
                break
        if placed < 0:
            placed = len(rem_chars)
            rem_chars.append(BIN_CHAR_CAP)
            rem_words.append(BIN_WORD_CAP)
            open_bins.append(placed)
        bin_of[j] = placed
        rem_chars[placed] -= c
        rem_words[placed] -= 1
        if rem_chars[placed] == 0 or rem_words[placed] == 0:
            open_bins.remove(placed)
    nbins = len(rem_chars)
    nbins = ((nbins + NCORES - 1) // NCORES) * NCORES
    sums = np.bincount(bin_of, weights=csorted, minlength=nbins)
    nword = np.bincount(bin_of, minlength=nbins)
    assert sums.max() <= BIN_CHAR_CAP and nword.max() <= BIN_WORD_CAP

    word_bin = np.full(n_words, -1, np.int32)
    word_bin[wsorted] = bin_of
    word_slot = np.full(n_words, -1, np.int32)
    ord2 = np.argsort(bin_of, kind="stable")
    slots = np.arange(nused) - np.concatenate([[0], np.cumsum(np.bincount(bin_of, minlength=nbins))])[bin_of[ord2]]
    word_slot[wsorted[ord2]] = slots
    return word_bin, word_slot, nbins, sums, nword


def _build_program(nch, tiles_per_chunk):
    ntiles = int(np.sum(tiles_per_chunk))
    f32 = mybir.dt.float32
    nc = bacc.Bacc("TRN2", target_bir_lowering=False, debug=False, num_devices=NCORES)
    weTb_ap = nc.dram_tensor("weTb", [nch, 128, WD], MM_DT, kind="ExternalInput").ap()
    ATb_ap = nc.dram_tensor("ATb", [8, 128, WD], MM_DT, kind="ExternalInput").ap()
    HBp_ap = nc.dram_tensor("HBp", [NE, WD], MM_DT, kind="ExternalInput").ap()
    # sel: [128, ntiles*128] (slot-row x char-col), oc: [NE, ntiles*128] — both
    # kept fully resident in SBUF, loaded once.
    sel_ap = nc.dram_tensor("sel", [128, ntiles * 128], MM_DT, kind="ExternalInput").ap()
    ocid_ap = nc.dram_tensor("ocid", [NE, ntiles * 128], MM_DT, kind="ExternalInput").ap()
    out_ap = nc.dram_tensor("out", [ntiles * 128, WD], f32, kind="ExternalOutput").ap()

    with tile.TileContext(nc) as tc:
        with tc.tile_pool(name="at", bufs=1) as atp, \
             tc.tile_pool(name="hb", bufs=1) as hbp_pool, \
             tc.tile_pool(name="sl", bufs=6) as selp, \
             tc.tile_pool(name="oc", bufs=6) as ocp, \
             tc.tile_pool(name="wet", bufs=6) as wetp, \
             tc.tile_pool(name="pb", bufs=4) as pbp, \
             tc.tile_pool(name="ob", bufs=5) as obp, \
             tc.tile_pool(name="ps_pre", bufs=2, space="PSUM") as pspre, \
             tc.tile_pool(name="ps_exp", bufs=6, space="PSUM") as psexp:
            # first chunk's weights first so the PE can start ASAP; AT k-slices
            # stream in the order the first k-chain consumes them.
            wb0 = wetp.tile([128, WD], MM_DT, tag="wet")
            nc.sync.dma_start(wb0[:], weTb_ap[0])
            at = atp.tile([128, 8 * WD], MM_DT)
            for k in range(8):
                nc.sync.dma_start(at[:, k * WD:(k + 1) * WD], ATb_ap[k])
            hb = hbp_pool.tile([NE, WD], MM_DT)
            nc.sync.dma_start(hb[:], HBp_ap[:])
            tg = 0
            for c in range(nch):
                if c == 0:
                    wb = wb0
                else:
                    wb = wetp.tile([128, WD], MM_DT, tag="wet")
                    nc.sync.dma_start(wb[:], weTb_ap[c])
                tpc = int(tiles_per_chunk[c])
                cs = tg * 128
                sl = selp.tile([128, tpc * 128], MM_DT, tag="sl")
                nc.sync.dma_start(sl[:], sel_ap[:, cs:cs + tpc * 128])
                oc = ocp.tile([NE, tpc * 128], MM_DT, tag="oc")
                nc.sync.dma_start(oc[:], ocid_ap[:, cs:cs + tpc * 128])
                P = pbp.tile([128, WD], MM_DT)
                for n in range(2):
                    pp = pspre.tile([128, 512], f32, space="PSUM")
                    for k in range(8):
                        nc.tensor.matmul(
                            pp[:],
                            wb[:, k * 128:(k + 1) * 128],
                            at[:, k * WD + n * 512: k * WD + (n + 1) * 512],
                            start=(k == 0), stop=(k == 7))
                    # alternate the PSUM->SBUF copy between DVE and ACT
                    if n == 0:
                        nc.vector.tensor_copy(P[:, n * 512:(n + 1) * 512], pp[:])
                    else:
                        nc.scalar.copy(P[:, n * 512:(n + 1) * 512], pp[:])
                ob = obp.tile([128, tpc * WD], f32, tag="ob")
                # batch expansion in pairs of tiles: 4 sel matmuls then 4 oc
                # matmuls — longer uninterrupted PE runs (fewer sem waits
                # between matmuls) while staying within 4 PSUM banks.
                for i0 in range(0, tpc, 3):
                    grp = range(i0, min(i0 + 3, tpc))
                    pes = {}
                    for i in grp:
                        for n in range(2):
                            pe = psexp.tile([128, 512], f32, space="PSUM")
                            pes[i, n] = pe
                            nc.tensor.matmul(pe[:], sl[:, i * 128:(i + 1) * 128],
                                             P[:, n * 512:(n + 1) * 512],
                                             start=True, stop=False)
                    for i in grp:
                        for n in range(2):
                            pe = pes[i, n]
                            nc.tensor.matmul(pe[:], oc[:, i * 128:(i + 1) * 128],
                                             hb[:, n * 512:(n + 1) * 512],
                                             start=False, stop=True)
                            dst = ob[:, i * WD + n * 512: i * WD + (n + 1) * 512]
                            # relu split across ACT and DVE to drain at PE pace
                            if n == 0:
                                nc.scalar.activation(
                                    dst, pe[:], mybir.ActivationFunctionType.Relu)
                            else:
                                nc.vector.tensor_scalar_max(dst, pe[:], 0.0)
                    tg += len(grp)
                # one merged store for the chunk's tiles:
                # DRAM rows [(tg-tpc)*128, tg*128) <- ob[p, i*WD+f]
                if c < nch - 1:
                    dram = out_ap[(tg - tpc) * 128: tg * 128, :].rearrange(
                        "(i p) f -> p i f", p=128)
                    nc.gpsimd.dma_start(dram, ob[:].rearrange("p (i f) -> p i f", f=WD))
                else:
                    # last chunk: store per tile so the final rows drain early
                    for i in range(tpc):
                        t0 = (tg - tpc + i) * 128
                        nc.gpsimd.dma_start(
                            out_ap[t0:t0 + 128, :],
                            ob[:, i * WD:(i + 1) * WD])
    nc.compile()
    return nc


def kernel(word_emb, char_ids, word_ids, E, W_ih, b_ih, b_hh, W_lin, b_lin,
           _timing=None, _trace_cores=None):
    word_emb = np.asarray(word_emb, np.float32)
    char_ids = np.asarray(char_ids, np.int32)
    word_ids = np.asarray(word_ids, np.int32)
    E = np.asarray(E, np.float32)
    W_ih = np.asarray(W_ih, np.float32)
    b_ih = np.asarray(b_ih, np.float32)
    b_hh = np.asarray(b_hh, np.float32)
    W_lin = np.asarray(W_lin, np.float32)
    b_lin = np.asarray(b_lin, np.float32)

    T = char_ids.shape[0]
    NW = word_emb.shape[0]

    HBp = _hb_table(E, W_ih, b_ih, b_hh, W_lin, b_lin)
    A = np.ascontiguousarray(W_lin[:, :WD])

    word_bin, word_slot, nbins, bin_chars, bin_words = _pack_bins(word_ids, NW)

    # deal bins to cores by descending char count: rank r -> core r % NCORES,
    # chunk r // NCORES; the rank ordering makes chunk char counts uniform
    # across cores so the shared tiles_per_chunk wastes little padding.
    rank_of_bin = np.empty(nbins, np.int64)
    rank_of_bin[np.argsort(-bin_chars, kind="stable")] = np.arange(nbins)
    core_of_bin = (rank_of_bin % NCORES).astype(np.int32)
    chunk_of_bin = (rank_of_bin // NCORES).astype(np.int32)
    nch = nbins // NCORES

    # chars sorted by (bin, slot) -> contiguous per bin, word-major inside
    cb = word_bin[word_ids]
    cslot = word_slot[word_ids]
    ckey = cb.astype(np.int64) * 1024 + cslot
    corder = np.argsort(ckey, kind="stable")
    per_bin = np.bincount(cb, minlength=nbins)
    bstart = np.concatenate([[0], np.cumsum(per_bin)])

    chunk_cnt = np.zeros((NCORES, nch), np.int64)
    for b in range(nbins):
        chunk_cnt[core_of_bin[b], chunk_of_bin[b]] = per_bin[b]
    while nch > 1 and chunk_cnt[:, nch - 1].max() == 0:
        nch -= 1
    tiles_per_chunk = np.maximum(1, np.ceil(chunk_cnt[:, :nch].max(axis=0) / 128).astype(np.int64))
    ntiles = int(tiles_per_chunk.sum())
    tile_base = np.concatenate([[0], np.cumsum(tiles_per_chunk)])

    ATb = np.ascontiguousarray(A.T.reshape(8, 128, WD)).astype(NP_DT)
    HBq = HBp.astype(NP_DT)
    in_maps = []
    origs = []
    for m in range(NCORES):
        weTb = np.zeros((nch, 128, WD), NP_DT)
        sel = np.zeros((128, ntiles * 128), NP_DT)
        ocid = np.zeros((NE, ntiles * 128), NP_DT)
        orig = np.full(ntiles * 128, -1, np.int64)
        for c in range(nch):
            bs = np.nonzero((core_of_bin == m) & (chunk_of_bin == c))[0]
            if len(bs) == 0:
                continue
            b = bs[0]
            lo, hi = bstart[b], bstart[b + 1]
            chars = corder[lo:hi]
            wlist = np.nonzero(word_bin == b)[0]
            wlist = wlist[np.argsort(word_slot[wlist])]
            nwb = len(wlist)
            if nwb:
                rows = word_emb[wlist]  # [nwb, WD]
                blk = rows.T.reshape(8, 128, nwb).transpose(1, 0, 2)
                weTb[c, :, :].reshape(128, 8, 128)[:, :, :nwb] = blk
            q = np.arange(len(chars))
            col = tile_base[c] * 128 + q
            sel[cslot[chars], col] = 1.0
            ocid[char_ids[chars], col] = 1.0
            orig[col] = chars
        in_maps.append({
            "weTb": weTb,
            "ATb": ATb,
            "HBp": HBq,
            "sel": sel,
            "ocid": ocid,
        })
        origs.append(orig)

    nc = _build_program(nch, tiles_per_chunk)
    kwargs = {}
    if _trace_cores is not None:
        kwargs = dict(trace=True, trace_cores=_trace_cores)
    res = run_bass_kernel_spmd(nc, in_maps, core_ids=list(range(NCORES)), **kwargs)
    if _timing is not None:
        _timing["exec_time_ns"] = res.exec_time_ns
        _timing["results"] = res

    out = np.empty((T, WD), np.float32)
    for m in range(NCORES):
        o = res.results[m]["out"]
        v = origs[m] >= 0
        out[origs[m][v]] = o[v]
    return out
